# revision 52
# baseline (speedup 1.0000x reference)
"""Trainium2 Bass kernel for nn_ExpertAdaRMSLayer (AdaRMS transformer layer).

Sharding: 8 cores = 4 batches (DP) x 2 token-halves. Each core computes its
1024 tokens end-to-end with no collectives; k/v (nkv=1) are computed
redundantly by the pair of cores sharing a batch. All activations are kept
feature-major [feature, token] on device; the host pre-transposes inputs /
weights and re-assembles the output. Columns are rolled per core so "own"
tokens are always columns 0..1023 (keeps the SPMD program uniform; attention
is permutation-invariant over keys).

v3: adaptive gains g1/g2 precomputed on host. Reciprocal paths via PE
ones-broadcast matmul + full-width DVE reciprocal. Stage order interleaves
the serial rms chains (ACT squares -> sqrt -> bcast -> recip -> STT) under
the dense matmul streams of the next stage: B(0) B(1) Wq(0) B(2) Wq(1) B(3)
Wk Wv rope, and o_proj is token-tile-outer so rms2(nt) hides under
o_proj(nt+1) and the MLP.
"""

import os
import sys
from contextlib import ExitStack

import numpy as np

sys.path.insert(0, "/opt/trn_rl_repo")

import ml_dtypes

import concourse.bass as bass
import concourse.mybir as mybir
import concourse.tile as tile

BF16 = ml_dtypes.bfloat16
F32 = np.float32

# Model dims (hardcoded per spec)
HIDDEN, NQ, NKV, HD, INTER = 2048, 8, 1, 256, 8192
B, S = 4, 2048
EPS = 1e-6
ROPE_BASE = 10000.0

P = 128
HC = HIDDEN // P          # 16 hidden chunks
IC = INTER // P           # 64 inter chunks
QC = (NQ * HD) // P       # 16 q-feature chunks
KC_HD = HD // P           # 2 head-dim chunks
T_OWN = S // 2            # 1024 own tokens per core
T_FULL = S                # 2048 tokens per batch
FD = 512                  # matmul free-dim tile (one PSUM bank of f32)
NT_OWN = T_OWN // FD      # 2
NT_FULL = T_FULL // FD    # 4
N_CORES = 8

DT = mybir.dt.bfloat16    # matmul operand dtype
DT_R = mybir.dt.float32r  # full-rate fp32 dtype for rms sum-of-squares
AF = mybir.ActivationFunctionType
ALU = mybir.AluOpType

_CACHE = {}
LAST_RESULTS = None


PADW = 128  # contiguous strip rows: kc/elem dims merge so strip DMAs ride
             # the large-packet DIRECT2D path (the old 132-pad forced 256B
             # generic packets, capping aggregate DMA at ~95GB/s; multi-wait
             # DMAs are handled by _split_dma_waits)


def _strips(WT, KC, MC):
    """WT: [K, M] f32 with rows = contraction dim. Returns bf16 array
    [MC, 128, KC, PADW] with [m][p][kc][:128] = WT[kc*128+p, m*128+j]."""
    K, M = WT.shape
    assert K == KC * P and M == MC * P
    A = WT.reshape(KC, P, MC, P).transpose(2, 1, 0, 3)
    out = np.zeros((MC, P, KC, PADW), dtype=BF16)
    out[:, :, :, :P] = A.astype(BF16)
    return out


def build_program():
    if "nc" in _CACHE:
        return _CACHE["nc"]

    nc = bass.Bass()
    dram = {}

    def inp(name, shape, dt):
        dram[name] = nc.declare_dram_parameter(name, list(shape), dt,
                                               isOutput=False)

    inp("xT", (HIDDEN, T_FULL), mybir.dt.float32)
    inp("xTb", (HIDDEN, T_FULL), DT)
    inp("cosT", (P, T_FULL), DT)
    inp("sinT", (P, T_FULL), DT)
    inp("g1", (P, HC), mybir.dt.float32)
    inp("g2", (P, HC), mybir.dt.float32)
    inp("wq", (QC, P, HC, PADW), DT)
    inp("wk", (KC_HD, P, HC, PADW), DT)
    inp("wv", (HC, P, HD), DT)
    inp("wo", (HC, P, QC, PADW), DT)
    inp("wg", (IC, P, HC, PADW), DT)
    inp("wu", (IC, P, HC, PADW), DT)
    inp("wd", (HC, P, IC, PADW), DT)
    outT = nc.declare_dram_parameter("outT", [HIDDEN, T_OWN],
                                     mybir.dt.float32, isOutput=True)
    if os.environ.get("KERNEL_DEBUG_DUMP"):
        for nm, shp in (("dbg_h1T", [HIDDEN, T_FULL]), ("dbg_qT", [NQ * HD, T_OWN]),
                        ("dbg_kT", [HD, T_FULL]), ("dbg_v", [T_FULL, HD]),
                        ("dbg_ctxT", [NQ * HD, T_OWN])):
            dram[nm] = nc.dram_tensor(nm, shp, mybir.dt.float32)
    res2T = nc.dram_tensor("res2T", [HIDDEN, T_OWN], DT)

    _build_kernel(nc, dram, outT, res2T)
    if not os.environ.get("KERNEL_NO_WAIT_SPLIT"):
        _split_dma_waits(nc)
    _CACHE["nc"] = nc
    return nc


def _split_dma_waits(nc):
    """This walrus encodes at most ONE sync-wait per instruction (the ISA
    EVENTS struct has a single wait slot and this build refuses to split).
    Hoist all waits of multi-wait instructions onto standalone
    event-semaphore instructions on the issuing engine/sequencer, which
    executes them in program order before the original instruction."""
    n = 0
    for f in nc.m.functions:
        for bb in f.blocks:
            out = []
            changed = False
            for inst in bb.instructions:
                si = inst.sync_info
                if si is not None and len(si.on_wait) > 1:
                    for w in si.on_wait:
                        ev = mybir.InstEventSemaphore(
                            name=f"{inst.name}_w{n}", ins=[], outs=[])
                        ev.engine = inst.engine
                        ev.sync_info = mybir.SyncInfo(on_wait=[w],
                                                      on_update=[])
                        out.append(ev)
                        n += 1
                    inst.sync_info = mybir.SyncInfo(
                        on_wait=[], on_update=list(si.on_update))
                    changed = True
                out.append(inst)
            if changed:
                bb.instructions[:] = out
    return n


def _build_kernel(nc, dram, outT, res2T):
    xT_v = dram["xT"][:, :].rearrange("(c p) t -> p c t", p=P)
    xTb_v = dram["xTb"][:, :].rearrange("(c p) t -> p c t", p=P)
    res2T_v = res2T[:, :].rearrange("(c p) t -> p c t", p=P)
    outT_v = outT[:, :].rearrange("(c p) t -> p c t", p=P)

    with tile.TileContext(nc) as tc, ExitStack() as top:
        const = top.enter_context(tc.tile_pool(name="const", bufs=1))
        psA = top.enter_context(tc.tile_pool(name="psA", bufs=6, space="PSUM"))
        psB = top.enter_context(tc.tile_pool(name="psB", bufs=2, space="PSUM"))

        ones_bf = const.tile([P, 1], DT)
        nc.vector.memset(ones_bf, 1.0)
        ones_rf = const.tile([P, 1], mybir.dt.float32, name="ones_rf")
        nc.vector.memset(ones_rf, 1.0)
        ones_r = ones_rf.bitcast(DT_R)
        ones_row_f = const.tile([1, P], mybir.dt.float32, name="ones_row_f")
        nc.vector.memset(ones_row_f, 1.0)
        ones_row_r = ones_row_f.bitcast(DT_R)
        g1 = const.tile([P, HC], mybir.dt.float32, name="g1")
        g2 = const.tile([P, HC], mybir.dt.float32, name="g2")
        nc.sync.dma_start(out=g1, in_=dram["g1"][:, :])
        nc.sync.dma_start(out=g2, in_=dram["g2"][:, :])
        eps_t = const.tile([1, 1], mybir.dt.float32, name="eps_t")
        nc.vector.memset(eps_t, EPS)
        # PE warm-up: dense dummy matmul burst at t=0 so HAM un-throttles
        # before the first real matmul stream arrives.
        wu_w = const.tile([P, P], DT, name="wu_w")
        nc.vector.memset(wu_w, 0.0)
        wu_x = const.tile([P, FD], DT, name="wu_x")
        nc.vector.memset(wu_x, 0.0)
        wu_ps = psA.tile([P, FD], mybir.dt.float32, tag="pmm", name="ps_wu")
        NWU = 48
        for i in range(NWU):
            nc.tensor.matmul(wu_ps, wu_w, wu_x,
                             start=(i == 0), stop=(i == NWU - 1))

        def recip_bcast(sq_row, rec_out, tag):
            """sq_row: SBUF [1, FD] f32r (already sqrt'ed or raw denom).
            Broadcasts across 128 partitions via a f32r ones-matmul, then
            full-width DVE reciprocal into rec_out (SBUF [P, FD] f32)."""
            bc = psB.tile([P, FD], mybir.dt.float32, tag="psmall",
                          name=f"ps_bc_{tag}")
            nc.tensor.matmul(bc, ones_row_r, sq_row,
                             start=True, stop=True)
            nc.vector.reciprocal(rec_out, bc)

        # ---------------- stage B: ada_rms1 -> h1 (bf16), one token-tile
        poolBC = tc.alloc_tile_pool(name="poolBC", bufs=1)
        h1 = [poolBC.tile([P, HC, FD], DT, name=f"h1_{nt}")
              for nt in range(NT_FULL)]
        sB = tc.alloc_tile_pool(name="stB", bufs=2)

        def b_dma(nt):
            sl = slice(nt * FD, (nt + 1) * FD)
            xt = sB.tile([P, HC, FD], DT, tag="xt", name="x_tile")
            for kc in range(HC):
                nc.gpsimd.dma_start(out=xt[:, kc, :], in_=xTb_v[:, kc, sl])
            return xt

        def b_square_thunks(xt, sqs):
            """Per-chunk square ops as thunks so their emission can be
            interleaved into another block's ACT stream (avoids a 16-op
            head-of-line burst in front of that block's PSUM evacuations)."""
            def mk(kc):
                def go():
                    sq = sB.tile([P, FD], DT, tag="sq", bufs=16, name="sq")
                    nc.scalar.activation(sq, xt[:, kc, :], AF.Square)
                    sqs.append(sq)
                return go
            return [mk(kc) for kc in range(HC)]

        def b_reduce(nt, xt, sqs):
            """Batch ssum matmuls (inputs already computed -> no PE FIFO
            stall), then rms tail and h1 writeback."""
            ssum = psB.tile([1, FD], mybir.dt.float32, tag="psmall",
                            name="ps_ss")
            for kc in range(HC):
                nc.tensor.matmul(ssum, ones_bf, sqs[kc],
                                 start=(kc == 0), stop=(kc == HC - 1))
            rr = sB.tile([1, FD], DT_R, tag="rr", name="rr")
            nc.scalar.activation(rr, ssum, AF.Sqrt, bias=eps_t,
                                 scale=1.0 / HIDDEN)
            rrb = sB.tile([P, FD], mybir.dt.float32, tag="rrb", name="rrb")
            recip_bcast(rr, rrb, "rms1")
            for kc in range(HC):
                nc.vector.scalar_tensor_tensor(
                    out=h1[nt][:, kc, :], in0=xt[:, kc, :],
                    scalar=g1[:, kc:kc + 1], in1=rrb,
                    op0=ALU.mult, op1=ALU.mult)

        # ---------------- stage C pieces: QKV + RoPE
        poolCD = tc.alloc_tile_pool(name="poolCD", bufs=1, side="right")
        qT = poolCD.tile([P, QC, T_OWN], DT, name="qT")
        kT = poolCD.tile([P, KC_HD, T_FULL], DT, name="kT")
        vtok = poolCD.tile([P, HC, HD], DT, name="vtok")
        sC = tc.alloc_tile_pool(name="stC", bufs=3)

        def emit_some(thunks, n):
            for _ in range(n):
                if thunks:
                    thunks.pop(0)()

        def wq_tile(nt, filler=None):
            sl = slice(nt * FD, (nt + 1) * FD)
            for m in range(QC):
                strip = sC.tile([P, HC, P], DT, tag="w", bufs=2, name="wq_strip")
                nc.sync.dma_start(out=strip, in_=dram["wq"][m][:, :, :P])
                ps = psA.tile([P, FD], mybir.dt.float32, tag="pmm",
                              name="ps_q")
                for kc in range(HC):
                    nc.tensor.matmul(ps, strip[:, kc, :], h1[nt][:, kc, :],
                                     start=(kc == 0), stop=(kc == HC - 1))
                nc.scalar.copy(out=qT[:, m, sl], in_=ps)
                if filler:
                    emit_some(filler, 1)

        def wk_tile(nts, filler=None):
            for m in range(KC_HD):
                for nt in nts:
                    strip = sC.tile([P, HC, P], DT, tag="w", bufs=2, name="wk_strip")
                    nc.sync.dma_start(out=strip, in_=dram["wk"][m][:, :, :P])
                    sl = slice(nt * FD, (nt + 1) * FD)
                    ps = psA.tile([P, FD], mybir.dt.float32, tag="pmm",
                                  name="ps_k")
                    for kc in range(HC):
                        nc.tensor.matmul(ps, strip[:, kc, :], h1[nt][:, kc, :],
                                         start=(kc == 0), stop=(kc == HC - 1))
                    nc.scalar.copy(out=kT[:, m, sl], in_=ps)
                    if filler:
                        emit_some(filler, 3)

        def wv_rope():
            # v token-major: [key-token-in-chunk, key-chunk, hd]
            wv_sb = sC.tile([P, HC, HD], DT, tag="wv", bufs=1, name="wv_sb")
            nc.sync.dma_start(out=wv_sb, in_=dram["wv"][:, :, :].rearrange("c p d -> p c d"))
            for tm in range(T_FULL // P):
                ps = psA.tile([P, HD], mybir.dt.float32, tag="pmm",
                              name="ps_v")
                nt, tin = (tm * P) // FD, (tm * P) % FD
                tsl = slice(tin, tin + P)
                for kc in range(HC):
                    nc.tensor.matmul(ps, h1[nt][:, kc, tsl], wv_sb[:, kc, :],
                                     start=(kc == 0), stop=(kc == HC - 1))
                nc.scalar.copy(out=vtok[:, tm, :], in_=ps)

            # RoPE (in-place on qT / kT); kT first so attention can start
            cos_f = sC.tile([P, T_FULL], DT, tag="cos", bufs=1, name="cos_f")
            sin_f = sC.tile([P, T_FULL], DT, tag="sin", bufs=1, name="sin_f")
            nc.gpsimd.dma_start(out=cos_f, in_=dram["cosT"][:, :])
            nc.gpsimd.dma_start(out=sin_f, in_=dram["sinT"][:, :])

            def rope_pair(u, v_, cos_t, sin_t, width):
                t1 = sC.tile([P, width], DT, tag="rt1", bufs=1, name="rope_t1")
                t2 = sC.tile([P, width], DT, tag="rt2", bufs=1, name="rope_t2")
                t3 = sC.tile([P, width], DT, tag="rt3", bufs=1, name="rope_t3")
                nc.vector.tensor_mul(t1, u, sin_t)      # u*sin
                nc.vector.tensor_mul(t2, u, cos_t)      # u*cos
                nc.vector.tensor_mul(t3, v_, sin_t)     # v*sin
                nc.vector.tensor_sub(u, t2, t3)         # u <- u*cos - v*sin
                nc.vector.tensor_mul(t2, v_, cos_t)     # v*cos
                nc.vector.tensor_add(v_, t2, t1)        # v <- v*cos + u*sin

            rope_pair(kT[:, 0, :], kT[:, 1, :], cos_f, sin_f, T_FULL)
            for h in range(NQ):
                rope_pair(qT[:, 2 * h, :], qT[:, 2 * h + 1, :],
                          cos_f[:, :T_OWN], sin_f[:, :T_OWN], T_OWN)
            if "dbg_h1T" in dram:
                dv = dram["dbg_h1T"][:, :].rearrange("(c p) t -> p c t", p=P)
                for nt in range(NT_FULL):
                    nc.gpsimd.dma_start(
                        out=dv[:, :, nt * FD:(nt + 1) * FD], in_=h1[nt])
                dv = dram["dbg_qT"][:, :].rearrange("(c p) t -> p c t", p=P)
                nc.gpsimd.dma_start(out=dv, in_=qT)
                dv = dram["dbg_kT"][:, :].rearrange("(c p) t -> p c t", p=P)
                nc.gpsimd.dma_start(out=dv, in_=kT)
                dv = dram["dbg_v"][:, :].rearrange("(c p) t -> p c t", p=P)
                nc.gpsimd.dma_start(out=dv, in_=vtok)

        # interleave: rms loads/squares overlap the Wq matmul blocks; the
        # batch ssum-reduce matmuls are emitted only after a dense block so
        # their inputs are ready when the PE FIFO reaches them. Squares of
        # tile nt+1 are interleaved into tile nt's evacuation stream on ACT.
        xt0 = b_dma(nt=0)
        sq0 = []
        emit_some(b_square_thunks(xt0, sq0), HC)
        b_reduce(0, xt0, sq0)
        xt1 = b_dma(1)
        sq1 = []
        wq_tile(0, filler=b_square_thunks(xt1, sq1))
        b_reduce(1, xt1, sq1)
        xt2 = b_dma(2)
        sq2 = []
        wq_tile(1, filler=b_square_thunks(xt2, sq2))
        b_reduce(2, xt2, sq2)
        xt3 = b_dma(3)
        sq3 = []
        wk_tile([0, 1, 2], filler=b_square_thunks(xt3, sq3))
        b_reduce(3, xt3, sq3)
        wk_tile([3])
        wv_rope()
        sC.release()
        sB.release()
        poolBC.release()

        # ---------------- stage D: attention (column softmax, no transposes)
        poolDE = tc.alloc_tile_pool(name="poolDE", bufs=1)
        ctxT = poolDE.tile([P, QC, T_OWN], DT, name="ctxT")
        with tc.tile_pool(name="stD", bufs=2) as sD:
            for h in range(NQ):
                attnT = sD.tile([P, HC, T_OWN], DT, tag="attn", name="attnT")
                for sm in range(T_FULL // P):
                    for nt in range(NT_OWN):
                        sl = slice(nt * FD, (nt + 1) * FD)
                        ps = psA.tile([P, FD], mybir.dt.float32, tag="pmm",
                                      name="ps_sc")
                        for dc in range(KC_HD):
                            nc.tensor.matmul(
                                ps, kT[:, dc, sm * P:(sm + 1) * P],
                                qT[:, 2 * h + dc, sl],
                                start=(dc == 0), stop=(dc == KC_HD - 1))
                        nc.scalar.activation(attnT[:, sm, sl], ps, AF.Exp,
                                             scale=1.0 / 16.0)
                for nt in range(NT_OWN):
                    sl = slice(nt * FD, (nt + 1) * FD)
                    cs = psB.tile([1, FD], mybir.dt.float32, tag="psmall",
                                  name="ps_cs")
                    for kc in range(HC):
                        nc.tensor.matmul(cs, ones_bf, attnT[:, kc, sl],
                                         start=(kc == 0), stop=(kc == HC - 1))
                    cs_sb = sD.tile([1, FD], DT_R, tag="cs_sb",
                                    name="cs_sb")
                    nc.scalar.copy(out=cs_sb, in_=cs)
                    recb = sD.tile([P, FD], mybir.dt.float32, tag="recb",
                                   name="recb")
                    # ctx dm=0 keeps PE busy while the ACT copy + bcast for
                    # the softmax denominator run; then dm=1.
                    pss = []
                    for dm in range(KC_HD):
                        if dm == 1:
                            recip_bcast(cs_sb, recb, "attn")
                        ps = psA.tile([P, FD], mybir.dt.float32, tag="pmm",
                                      name="ps_ctx")
                        for kc in range(HC):
                            nc.tensor.matmul(
                                ps, vtok[:, kc, dm * P:(dm + 1) * P],
                                attnT[:, kc, sl],
                                start=(kc == 0), stop=(kc == HC - 1))
                        pss.append(ps)
                    for dm in range(KC_HD):
                        nc.vector.tensor_mul(ctxT[:, 2 * h + dm, sl],
                                             pss[dm], recb)
            if "dbg_ctxT" in dram:
                dv = dram["dbg_ctxT"][:, :].rearrange("(c p) t -> p c t", p=P)
                nc.gpsimd.dma_start(out=dv, in_=ctxT)
        poolCD.release()

        # ---------------- stage E: o_proj + residual + ada_rms2 -> h2
        # token-tile-outer so the rms2(nt) chain hides under o_proj(nt+1)
        # and the MLP matmul stream.
        poolEF = tc.alloc_tile_pool(name="poolEF", bufs=1, side="right")
        h2 = [poolEF.tile([P, HC, FD], DT, name=f"h2_{nt}")
              for nt in range(NT_OWN)]
        sE = tc.alloc_tile_pool(name="stE", bufs=2, side="right")

        def stage_e(nt, pre_strips=None, next_pre=None):
            sl = slice(nt * FD, (nt + 1) * FD)
            res2 = sE.tile([P, HC, FD], DT, tag="res2", bufs=1,
                           name=f"res2_{nt}")
            sqs = []
            for m in range(HC):
                if pre_strips is not None and m < len(pre_strips):
                    strip = pre_strips[m]
                else:
                    strip = sE.tile([P, QC, P], DT, tag="w", bufs=2,
                                    name="wo_strip")
                    nc.sync.dma_start(out=strip, in_=dram["wo"][m][:, :, :P])
                ps = psA.tile([P, FD], mybir.dt.float32, tag="pmm",
                              name="ps_o")
                for kc in range(QC):
                    nc.tensor.matmul(ps, strip[:, kc, :], ctxT[:, kc, sl],
                                     start=(kc == 0), stop=(kc == QC - 1))
                xo = sE.tile([P, FD], mybir.dt.float32, tag="xo", bufs=2,
                             name="xo")
                nc.gpsimd.dma_start(out=xo, in_=xT_v[:, m, sl])
                nc.vector.scalar_tensor_tensor(
                    out=res2[:, m, :], in0=ps, scalar=0.0,
                    in1=xo, op0=ALU.bypass, op1=ALU.add)
                sq = sE.tile([P, FD], DT, tag="sq", bufs=16, name="sq2")
                nc.scalar.activation(sq, res2[:, m, :], AF.Square)
                sqs.append(sq)
                if next_pre is not None and m == HC - 2:
                    # prefetch the first o_proj strips of the next token tile
                    for j, t in enumerate(next_pre):
                        nc.sync.dma_start(out=t, in_=dram["wo"][j][:, :, :P])
            ssum = psB.tile([1, FD], mybir.dt.float32, tag="psmall",
                            name="ps_ss2")
            for m in range(HC):
                nc.tensor.matmul(ssum, ones_bf, sqs[m],
                                 start=(m == 0), stop=(m == HC - 1))
            rr = sE.tile([1, FD], DT_R, tag="rr", name="rr2")
            nc.scalar.activation(rr, ssum, AF.Sqrt, bias=eps_t,
                                 scale=1.0 / HIDDEN)
            rrb = sE.tile([P, FD], mybir.dt.float32, tag="rrb", name="rrb2")
            recip_bcast(rr, rrb, "rms2")
            for kc in range(HC):
                nc.vector.scalar_tensor_tensor(
                    out=h2[nt][:, kc, :], in0=res2[:, kc, :],
                    scalar=g2[:, kc:kc + 1], in1=rrb,
                    op0=ALU.mult, op1=ALU.mult)
            nc.gpsimd.dma_start(out=res2T_v[:, :, sl], in_=res2)

        # ---------------- stage F: SwiGLU MLP + final residual
        def f_strip(w, im, q="sync"):
            s = sF.tile([P, HC, P], DT, tag="w", bufs=3,
                        name=f"{w}_strip")
            eng = nc.sync if q == "sync" else nc.gpsimd
            eng.dma_start(out=s, in_=dram[w][im][:, :, :P])
            return s

        def f_gate_up(tt, pre):
            sl = slice(tt * FD, (tt + 1) * FD)
            act = sF.tile([P, IC, FD], DT, tag="act", bufs=1, name="act")
            for im in range(IC):
                if pre is not None and im == 0:
                    gstrip, ustrip = pre
                else:
                    gstrip = f_strip("wg", im)
                    ustrip = f_strip("wu", im, q="gpsimd")
                ps_g = psA.tile([P, FD], mybir.dt.float32, tag="pmm",
                                name="ps_g")
                for kc in range(HC):
                    nc.tensor.matmul(ps_g, gstrip[:, kc, :],
                                     h2[tt][:, kc, :],
                                     start=(kc == 0), stop=(kc == HC - 1))
                sil = sF.tile([P, FD], mybir.dt.float32, tag="sil",
                              name="sil")
                nc.scalar.activation(sil, ps_g, AF.Sigmoid)
                nc.vector.tensor_mul(sil, sil, ps_g)
                ps_u = psA.tile([P, FD], mybir.dt.float32, tag="pmm",
                                name="ps_u")
                for kc in range(HC):
                    nc.tensor.matmul(ps_u, ustrip[:, kc, :],
                                     h2[tt][:, kc, :],
                                     start=(kc == 0), stop=(kc == HC - 1))
                nc.vector.tensor_mul(act[:, im, :], sil, ps_u)
            return act

        def f_down(tt, act):
            sl = slice(tt * FD, (tt + 1) * FD)
            for dm in range(HC):
                dstrip = sF.tile([P, IC, P], DT, tag="wd", bufs=2,
                                 name="wd_strip")
                nc.gpsimd.dma_start(out=dstrip, in_=dram["wd"][dm][:, :, :P])
                ps_d = psA.tile([P, FD], mybir.dt.float32, tag="pmm",
                                name="ps_d")
                for kc in range(IC):
                    nc.tensor.matmul(ps_d, dstrip[:, kc, :],
                                     act[:, kc, :],
                                     start=(kc == 0), stop=(kc == IC - 1))
                r2c = sF.tile([P, FD], DT, tag="r2c", bufs=1, name="r2c")
                nc.sync.dma_start(out=r2c, in_=res2T_v[:, dm, sl])
                ot = sF.tile([P, FD], mybir.dt.float32, tag="ot",
                             name="ot")
                nc.vector.tensor_add(ot, ps_d, r2c)
                nc.sync.dma_start(out=outT_v[:, dm, sl], in_=ot)

        pre_e1 = [sE.tile([P, QC, P], DT, tag="wopre", bufs=1,
                          name="wo_pre0")]
        stage_e(0, next_pre=pre_e1)
        stage_e(1, pre_strips=pre_e1)
        poolDE.release()
        sF = tc.alloc_tile_pool(name="stF", bufs=2, side="right")
        act0 = f_gate_up(0, None)
        # emit next tile's first gate/up strip loads before the down loop so
        # they sit ahead of the r2c loads in the sync DMA FIFO
        pre1 = (f_strip("wg", 0), f_strip("wu", 0))
        f_down(0, act0)
        act1 = f_gate_up(1, pre1)
        f_down(1, act1)
        sF.release()
        sE.release()
        poolEF.release()


def _prep_inputs(x, pos_ids, time_emb, ln1_w, ln1_tw, ln2_w, ln2_tw,
                 Wq, Wk, Wv, Wo, Wg, Wu, Wd):
    """Host-side layout prep. Returns list of per-core in_maps."""
    shared = {
        "wq": _strips(Wq.T, HC, QC),
        "wk": _strips(Wk.T, HC, KC_HD),
        "wv": np.ascontiguousarray(Wv.T.reshape(HC, P, HD)).astype(BF16),
        "wo": _strips(Wo.T, QC, HC),
        "wg": _strips(Wg.T, HC, IC),
        "wu": _strips(Wu.T, HC, IC),
        "wd": _strips(Wd.T, IC, HC),
    }
    inv_freq = 1.0 / (ROPE_BASE **
                      (np.arange(0, HD, 2, dtype=np.float64) / HD))
    # adaptive gains on host (tiny matvec): g = w * (1 + t @ tw.T)
    t_all = np.asarray(time_emb).astype(F32)                    # (B, H)
    gain1 = (np.asarray(ln1_w) * (1.0 + t_all @ np.asarray(ln1_tw).T.astype(F32)))
    gain2 = (np.asarray(ln2_w) * (1.0 + t_all @ np.asarray(ln2_tw).T.astype(F32)))
    in_maps = []
    for c in range(N_CORES):
        b, half = c // 2, c % 2
        perm = np.r_[np.arange(half * T_OWN, (half + 1) * T_OWN),
                     np.arange((1 - half) * T_OWN, (2 - half) * T_OWN)]
        xTb = np.ascontiguousarray(np.asarray(x[b]).T[:, perm]).astype(F32)
        ang = (np.asarray(pos_ids[b])[perm].astype(np.float64)[:, None]
               * inv_freq[None, :])
        m = dict(shared)
        m["xT"] = xTb
        m["xTb"] = xTb.astype(BF16)
        m["cosT"] = np.ascontiguousarray(np.cos(ang).T).astype(BF16)
        m["sinT"] = np.ascontiguousarray(np.sin(ang).T).astype(BF16)
        m["g1"] = np.ascontiguousarray(gain1[b].reshape(HC, P).T).astype(F32)
        m["g2"] = np.ascontiguousarray(gain2[b].reshape(HC, P).T).astype(F32)
        in_maps.append(m)
    return in_maps


def kernel(**inputs):
    global LAST_RESULTS
    from concourse.bass_utils import run_bass_kernel_spmd

    nc = build_program()
    in_maps = _prep_inputs(**{k: np.asarray(v) for k, v in inputs.items()})
    trace = bool(int(os.environ.get("KERNEL_TRACE", "0")))
    kw = {}
    if os.environ.get("KERNEL_TMPDIR"):
        os.makedirs(os.environ["KERNEL_TMPDIR"], exist_ok=True)
        kw["tmpdir"] = os.environ["KERNEL_TMPDIR"]
    res = run_bass_kernel_spmd(nc, in_maps, core_ids=list(range(N_CORES)),
                               trace=trace, **kw)
    LAST_RESULTS = res
    out = np.empty((B, S, HIDDEN), dtype=F32)
    for c in range(N_CORES):
        b, half = c // 2, c % 2
        out[b, half * T_OWN:(half + 1) * T_OWN, :] = res.results[c]["outT"].T
    return out


# revision 57
# speedup vs baseline: 1.0503x; 1.0503x over previous
"""Trainium2 Bass kernel for nn_ExpertAdaRMSLayer (AdaRMS transformer layer).

Sharding: 8 cores = 4 batches (DP) x 2 token-halves. Each core computes its
1024 tokens end-to-end with no collectives; k/v (nkv=1) are computed
redundantly by the pair of cores sharing a batch. All activations are kept
feature-major [feature, token] on device; the host pre-transposes inputs /
weights and re-assembles the output. Columns are rolled per core so "own"
tokens are always columns 0..1023 (keeps the SPMD program uniform; attention
is permutation-invariant over keys).

v3: adaptive gains g1/g2 precomputed on host. Reciprocal paths via PE
ones-broadcast matmul + full-width DVE reciprocal. Stage order interleaves
the serial rms chains (ACT squares -> sqrt -> bcast -> recip -> STT) under
the dense matmul streams of the next stage: B(0) B(1) Wq(0) B(2) Wq(1) B(3)
Wk Wv rope, and o_proj is token-tile-outer so rms2(nt) hides under
o_proj(nt+1) and the MLP.
"""

import os
import sys
from contextlib import ExitStack

import numpy as np

sys.path.insert(0, "/opt/trn_rl_repo")

import ml_dtypes

import concourse.bass as bass
import concourse.mybir as mybir
import concourse.tile as tile

BF16 = ml_dtypes.bfloat16
F32 = np.float32

# Model dims (hardcoded per spec)
HIDDEN, NQ, NKV, HD, INTER = 2048, 8, 1, 256, 8192
B, S = 4, 2048
EPS = 1e-6
ROPE_BASE = 10000.0

P = 128
HC = HIDDEN // P          # 16 hidden chunks
IC = INTER // P           # 64 inter chunks
QC = (NQ * HD) // P       # 16 q-feature chunks
KC_HD = HD // P           # 2 head-dim chunks
T_OWN = S // 2            # 1024 own tokens per core
T_FULL = S                # 2048 tokens per batch
FD = 512                  # matmul free-dim tile (one PSUM bank of f32)
NT_OWN = T_OWN // FD      # 2
NT_FULL = T_FULL // FD    # 4
N_CORES = 8

DT = mybir.dt.bfloat16    # matmul operand dtype
DT_R = mybir.dt.float32r  # full-rate fp32 dtype for rms sum-of-squares
AF = mybir.ActivationFunctionType
ALU = mybir.AluOpType

_CACHE = {}
LAST_RESULTS = None


PADW = 128  # contiguous strip rows: kc/elem dims merge so strip DMAs ride
             # the large-packet DIRECT2D path (the old 132-pad forced 256B
             # generic packets, capping aggregate DMA at ~95GB/s; multi-wait
             # DMAs are handled by _split_dma_waits)


def _strips(WT, KC, MC):
    """WT: [K, M] f32 with rows = contraction dim. Returns bf16 array
    [MC, 128, KC, PADW] with [m][p][kc][:128] = WT[kc*128+p, m*128+j]."""
    K, M = WT.shape
    assert K == KC * P and M == MC * P
    A = WT.reshape(KC, P, MC, P).transpose(2, 1, 0, 3)
    out = np.zeros((MC, P, KC, PADW), dtype=BF16)
    out[:, :, :, :P] = A.astype(BF16)
    return out


def build_program():
    if "nc" in _CACHE:
        return _CACHE["nc"]

    nc = bass.Bass()
    dram = {}

    def inp(name, shape, dt):
        dram[name] = nc.declare_dram_parameter(name, list(shape), dt,
                                               isOutput=False)

    inp("xT", (HIDDEN, T_FULL), mybir.dt.float32)
    inp("xTb", (HIDDEN, T_FULL), DT)
    inp("cosT", (P, T_FULL), DT)
    inp("sinT", (P, T_FULL), DT)
    inp("g1", (P, HC), mybir.dt.float32)
    inp("g2", (P, HC), mybir.dt.float32)
    inp("wq", (QC, P, HC, PADW), DT)
    inp("wk", (KC_HD, P, HC, PADW), DT)
    inp("wv", (HC, P, HD), DT)
    inp("wo", (HC, P, QC, PADW), DT)
    inp("wg", (IC, P, HC, PADW), DT)
    inp("wu", (IC, P, HC, PADW), DT)
    inp("wd", (HC, P, IC, PADW), DT)
    outT = nc.declare_dram_parameter("outT", [HIDDEN, T_OWN],
                                     mybir.dt.float32, isOutput=True)
    if os.environ.get("KERNEL_DEBUG_DUMP"):
        for nm, shp in (("dbg_h1T", [HIDDEN, T_FULL]), ("dbg_qT", [NQ * HD, T_OWN]),
                        ("dbg_kT", [HD, T_FULL]), ("dbg_v", [T_FULL, HD]),
                        ("dbg_ctxT", [NQ * HD, T_OWN])):
            dram[nm] = nc.dram_tensor(nm, shp, mybir.dt.float32)
    res2T = nc.dram_tensor("res2T", [HIDDEN, T_OWN], DT)

    _build_kernel(nc, dram, outT, res2T)
    if not os.environ.get("KERNEL_NO_WAIT_SPLIT"):
        _split_dma_waits(nc)
    _CACHE["nc"] = nc
    return nc


def _split_dma_waits(nc):
    """This walrus encodes at most ONE sync-wait per instruction (the ISA
    EVENTS struct has a single wait slot and this build refuses to split).
    Hoist all waits of multi-wait instructions onto standalone
    event-semaphore instructions on the issuing engine/sequencer, which
    executes them in program order before the original instruction."""
    n = 0
    for f in nc.m.functions:
        for bb in f.blocks:
            out = []
            changed = False
            for inst in bb.instructions:
                si = inst.sync_info
                if si is not None and len(si.on_wait) > 1:
                    for w in si.on_wait:
                        ev = mybir.InstEventSemaphore(
                            name=f"{inst.name}_w{n}", ins=[], outs=[])
                        ev.engine = inst.engine
                        ev.sync_info = mybir.SyncInfo(on_wait=[w],
                                                      on_update=[])
                        out.append(ev)
                        n += 1
                    inst.sync_info = mybir.SyncInfo(
                        on_wait=[], on_update=list(si.on_update))
                    changed = True
                out.append(inst)
            if changed:
                bb.instructions[:] = out
    return n


def _build_kernel(nc, dram, outT, res2T):
    xT_v = dram["xT"][:, :].rearrange("(c p) t -> p c t", p=P)
    xTb_v = dram["xTb"][:, :].rearrange("(c p) t -> p c t", p=P)
    res2T_v = res2T[:, :].rearrange("(c p) t -> p c t", p=P)
    outT_v = outT[:, :].rearrange("(c p) t -> p c t", p=P)

    with tile.TileContext(nc) as tc, ExitStack() as top:
        const = top.enter_context(tc.tile_pool(name="const", bufs=1))
        psA = top.enter_context(tc.tile_pool(name="psA", bufs=6, space="PSUM"))
        psB = top.enter_context(tc.tile_pool(name="psB", bufs=2, space="PSUM"))

        ones_bf = const.tile([P, 1], DT)
        nc.vector.memset(ones_bf, 1.0)
        ones_rf = const.tile([P, 1], mybir.dt.float32, name="ones_rf")
        nc.vector.memset(ones_rf, 1.0)
        ones_r = ones_rf.bitcast(DT_R)
        ones_row_f = const.tile([1, P], mybir.dt.float32, name="ones_row_f")
        nc.vector.memset(ones_row_f, 1.0)
        ones_row_r = ones_row_f.bitcast(DT_R)
        g1 = const.tile([P, HC], mybir.dt.float32, name="g1")
        g2 = const.tile([P, HC], mybir.dt.float32, name="g2")
        nc.sync.dma_start(out=g1, in_=dram["g1"][:, :])
        nc.sync.dma_start(out=g2, in_=dram["g2"][:, :])
        eps_t = const.tile([1, 1], mybir.dt.float32, name="eps_t")
        nc.vector.memset(eps_t, EPS)
        # PE warm-up: dense dummy matmul burst at t=0 so HAM un-throttles
        # before the first real matmul stream arrives.
        wu_w = const.tile([P, P], DT, name="wu_w")
        nc.vector.memset(wu_w, 0.0)
        wu_x = const.tile([P, FD], DT, name="wu_x")
        nc.vector.memset(wu_x, 0.0)
        wu_ps = psA.tile([P, FD], mybir.dt.float32, tag="pmm", name="ps_wu")
        NWU = 48
        for i in range(NWU):
            nc.tensor.matmul(wu_ps, wu_w, wu_x,
                             start=(i == 0), stop=(i == NWU - 1))

        def recip_bcast(sq_row, rec_out, tag):
            """sq_row: SBUF [1, FD] f32r (already sqrt'ed or raw denom).
            Broadcasts across 128 partitions via a f32r ones-matmul, then
            full-width DVE reciprocal into rec_out (SBUF [P, FD] f32)."""
            bc = psB.tile([P, FD], mybir.dt.float32, tag="psmall",
                          name=f"ps_bc_{tag}")
            nc.tensor.matmul(bc, ones_row_r, sq_row,
                             start=True, stop=True)
            nc.vector.reciprocal(rec_out, bc)

        # ---------------- stage B: ada_rms1 -> h1 (bf16), one token-tile
        poolBC = tc.alloc_tile_pool(name="poolBC", bufs=1)
        h1 = [poolBC.tile([P, HC, FD], DT, name=f"h1_{nt}")
              for nt in range(NT_FULL)]
        sB = tc.alloc_tile_pool(name="stB", bufs=2)

        def b_dma(nt):
            sl = slice(nt * FD, (nt + 1) * FD)
            xt = sB.tile([P, HC, FD], DT, tag="xt", name="x_tile")
            for kc in range(HC):
                nc.gpsimd.dma_start(out=xt[:, kc, :], in_=xTb_v[:, kc, sl])
            return xt

        def b_square_thunks(xt, sqs):
            """Per-chunk square ops as thunks so their emission can be
            interleaved into another block's ACT stream (avoids a 16-op
            head-of-line burst in front of that block's PSUM evacuations)."""
            def mk(kc):
                def go():
                    sq = sB.tile([P, FD], DT, tag="sq", bufs=13, name="sq")
                    nc.scalar.activation(sq, xt[:, kc, :], AF.Square)
                    sqs.append(sq)
                return go
            return [mk(kc) for kc in range(HC)]

        def b_reduce(nt, xt, sqs):
            """Batch ssum matmuls (inputs already computed -> no PE FIFO
            stall), then rms tail and h1 writeback."""
            ssum = psB.tile([1, FD], mybir.dt.float32, tag="psmall",
                            name="ps_ss")
            for kc in range(HC):
                nc.tensor.matmul(ssum, ones_bf, sqs[kc],
                                 start=(kc == 0), stop=(kc == HC - 1))
            rr = sB.tile([1, FD], DT_R, tag="rr", name="rr")
            nc.scalar.activation(rr, ssum, AF.Sqrt, bias=eps_t,
                                 scale=1.0 / HIDDEN)
            rrb = sB.tile([P, FD], mybir.dt.float32, tag="rrb", name="rrb")
            recip_bcast(rr, rrb, "rms1")
            for kc in range(HC):
                nc.vector.scalar_tensor_tensor(
                    out=h1[nt][:, kc, :], in0=xt[:, kc, :],
                    scalar=g1[:, kc:kc + 1], in1=rrb,
                    op0=ALU.mult, op1=ALU.mult)

        # ---------------- stage C pieces: QKV + RoPE
        poolCD = tc.alloc_tile_pool(name="poolCD", bufs=1, side="right")
        qT = poolCD.tile([P, QC, T_OWN], DT, name="qT")
        kT = poolCD.tile([P, KC_HD, T_FULL], DT, name="kT")
        vtok = poolCD.tile([P, HC, HD], DT, name="vtok")
        sC = tc.alloc_tile_pool(name="stC", bufs=3)

        def emit_some(thunks, n):
            for _ in range(n):
                if thunks:
                    thunks.pop(0)()

        def wq_tile(nt, filler=None):
            sl = slice(nt * FD, (nt + 1) * FD)
            for m in range(QC):
                strip = sC.tile([P, HC, P], DT, tag="w", bufs=3, name="wq_strip")
                eng = nc.sync if m % 2 == 0 else nc.gpsimd
                eng.dma_start(out=strip, in_=dram["wq"][m][:, :, :P])
                ps = psA.tile([P, FD], mybir.dt.float32, tag="pmm",
                              name="ps_q")
                for kc in range(HC):
                    nc.tensor.matmul(ps, strip[:, kc, :], h1[nt][:, kc, :],
                                     start=(kc == 0), stop=(kc == HC - 1))
                nc.scalar.copy(out=qT[:, m, sl], in_=ps)
                if filler:
                    emit_some(filler, 1)

        def wk_tile(nts, filler=None):
            for m in range(KC_HD):
                for nt in nts:
                    strip = sC.tile([P, HC, P], DT, tag="w", bufs=3, name="wk_strip")
                    eng = nc.sync if nt % 2 == 0 else nc.gpsimd
                    eng.dma_start(out=strip, in_=dram["wk"][m][:, :, :P])
                    sl = slice(nt * FD, (nt + 1) * FD)
                    ps = psA.tile([P, FD], mybir.dt.float32, tag="pmm",
                                  name="ps_k")
                    for kc in range(HC):
                        nc.tensor.matmul(ps, strip[:, kc, :], h1[nt][:, kc, :],
                                         start=(kc == 0), stop=(kc == HC - 1))
                    nc.scalar.copy(out=kT[:, m, sl], in_=ps)
                    if filler:
                        emit_some(filler, 3)

        def wv_rope():
            # v token-major: [key-token-in-chunk, key-chunk, hd]
            wv_sb = poolCD.tile([P, HC, HD], DT, name="wv_sb")
            nc.sync.dma_start(out=wv_sb, in_=dram["wv"][:, :, :].rearrange("c p d -> p c d"))
            for tm in range(T_FULL // P):
                ps = psA.tile([P, HD], mybir.dt.float32, tag="pmm",
                              name="ps_v")
                nt, tin = (tm * P) // FD, (tm * P) % FD
                tsl = slice(tin, tin + P)
                for kc in range(HC):
                    nc.tensor.matmul(ps, h1[nt][:, kc, tsl], wv_sb[:, kc, :],
                                     start=(kc == 0), stop=(kc == HC - 1))
                nc.scalar.copy(out=vtok[:, tm, :], in_=ps)

            # RoPE (in-place on qT / kT); kT first so attention can start
            cos_f = poolCD.tile([P, T_FULL], DT, name="cos_f")
            sin_f = poolCD.tile([P, T_FULL], DT, name="sin_f")
            nc.gpsimd.dma_start(out=cos_f, in_=dram["cosT"][:, :])
            nc.gpsimd.dma_start(out=sin_f, in_=dram["sinT"][:, :])

            def rope_pair(u, v_, cos_t, sin_t, width):
                t1 = sC.tile([P, width], DT, tag="rt1", bufs=1, name="rope_t1")
                t2 = sC.tile([P, width], DT, tag="rt2", bufs=1, name="rope_t2")
                t3 = sC.tile([P, width], DT, tag="rt3", bufs=1, name="rope_t3")
                nc.vector.tensor_mul(t1, u, sin_t)      # u*sin
                nc.vector.tensor_mul(t2, u, cos_t)      # u*cos
                nc.vector.tensor_mul(t3, v_, sin_t)     # v*sin
                nc.vector.tensor_sub(u, t2, t3)         # u <- u*cos - v*sin
                nc.vector.tensor_mul(t2, v_, cos_t)     # v*cos
                nc.vector.tensor_add(v_, t2, t1)        # v <- v*cos + u*sin

            rope_pair(kT[:, 0, :], kT[:, 1, :], cos_f, sin_f, T_FULL)
            for h in range(NQ):
                rope_pair(qT[:, 2 * h, :], qT[:, 2 * h + 1, :],
                          cos_f[:, :T_OWN], sin_f[:, :T_OWN], T_OWN)
            if "dbg_h1T" in dram:
                dv = dram["dbg_h1T"][:, :].rearrange("(c p) t -> p c t", p=P)
                for nt in range(NT_FULL):
                    nc.gpsimd.dma_start(
                        out=dv[:, :, nt * FD:(nt + 1) * FD], in_=h1[nt])
                dv = dram["dbg_qT"][:, :].rearrange("(c p) t -> p c t", p=P)
                nc.gpsimd.dma_start(out=dv, in_=qT)
                dv = dram["dbg_kT"][:, :].rearrange("(c p) t -> p c t", p=P)
                nc.gpsimd.dma_start(out=dv, in_=kT)
                dv = dram["dbg_v"][:, :].rearrange("(c p) t -> p c t", p=P)
                nc.gpsimd.dma_start(out=dv, in_=vtok)

        # interleave: rms loads/squares overlap the Wq matmul blocks; the
        # batch ssum-reduce matmuls are emitted only after a dense block so
        # their inputs are ready when the PE FIFO reaches them. Squares of
        # tile nt+1 are interleaved into tile nt's evacuation stream on ACT.
        xt0 = b_dma(nt=0)
        sq0 = []
        emit_some(b_square_thunks(xt0, sq0), HC)
        b_reduce(0, xt0, sq0)
        xt1 = b_dma(1)
        sq1 = []
        wq_tile(0, filler=b_square_thunks(xt1, sq1))
        b_reduce(1, xt1, sq1)
        xt2 = b_dma(2)
        sq2 = []
        wq_tile(1, filler=b_square_thunks(xt2, sq2))
        b_reduce(2, xt2, sq2)
        xt3 = b_dma(3)
        sq3 = []
        wk_tile([0, 1, 2], filler=b_square_thunks(xt3, sq3))
        b_reduce(3, xt3, sq3)
        wk_tile([3])
        wv_rope()
        sC.release()
        sB.release()
        poolBC.release()

        # ---------------- stage D: attention (column softmax, no transposes)
        poolDE = tc.alloc_tile_pool(name="poolDE", bufs=1)
        ctxT = poolDE.tile([P, QC, T_OWN], DT, name="ctxT")
        with tc.tile_pool(name="stD", bufs=2) as sD:
            for h in range(NQ):
                attnT = sD.tile([P, HC, T_OWN], DT, tag="attn", name="attnT")
                for sm in range(T_FULL // P):
                    for nt in range(NT_OWN):
                        sl = slice(nt * FD, (nt + 1) * FD)
                        ps = psA.tile([P, FD], mybir.dt.float32, tag="pmm",
                                      name="ps_sc")
                        for dc in range(KC_HD):
                            nc.tensor.matmul(
                                ps, kT[:, dc, sm * P:(sm + 1) * P],
                                qT[:, 2 * h + dc, sl],
                                start=(dc == 0), stop=(dc == KC_HD - 1))
                        nc.scalar.activation(attnT[:, sm, sl], ps, AF.Exp,
                                             scale=1.0 / 16.0)
                for nt in range(NT_OWN):
                    sl = slice(nt * FD, (nt + 1) * FD)
                    cs = psB.tile([1, FD], mybir.dt.float32, tag="psmall",
                                  name="ps_cs")
                    for kc in range(HC):
                        nc.tensor.matmul(cs, ones_bf, attnT[:, kc, sl],
                                         start=(kc == 0), stop=(kc == HC - 1))
                    cs_sb = sD.tile([1, FD], DT_R, tag="cs_sb",
                                    name="cs_sb")
                    nc.scalar.copy(out=cs_sb, in_=cs)
                    recb = sD.tile([P, FD], mybir.dt.float32, tag="recb",
                                   name="recb")
                    # ctx dm=0 keeps PE busy while the ACT copy + bcast for
                    # the softmax denominator run; then dm=1.
                    pss = []
                    for dm in range(KC_HD):
                        if dm == 1:
                            recip_bcast(cs_sb, recb, "attn")
                        ps = psA.tile([P, FD], mybir.dt.float32, tag="pmm",
                                      name="ps_ctx")
                        for kc in range(HC):
                            nc.tensor.matmul(
                                ps, vtok[:, kc, dm * P:(dm + 1) * P],
                                attnT[:, kc, sl],
                                start=(kc == 0), stop=(kc == HC - 1))
                        pss.append(ps)
                    for dm in range(KC_HD):
                        nc.vector.tensor_mul(ctxT[:, 2 * h + dm, sl],
                                             pss[dm], recb)
            if "dbg_ctxT" in dram:
                dv = dram["dbg_ctxT"][:, :].rearrange("(c p) t -> p c t", p=P)
                nc.gpsimd.dma_start(out=dv, in_=ctxT)
        poolCD.release()

        # ---------------- stage E: o_proj + residual + ada_rms2 -> h2
        # token-tile-outer so the rms2(nt) chain hides under o_proj(nt+1)
        # and the MLP matmul stream.
        poolEF = tc.alloc_tile_pool(name="poolEF", bufs=1, side="right")
        h2 = [poolEF.tile([P, HC, FD], DT, name=f"h2_{nt}")
              for nt in range(NT_OWN)]
        sE = tc.alloc_tile_pool(name="stE", bufs=2, side="right")

        def stage_e(nt, pre_strips=None, next_pre=None):
            sl = slice(nt * FD, (nt + 1) * FD)
            res2 = sE.tile([P, HC, FD], DT, tag="res2", bufs=1,
                           name=f"res2_{nt}")
            sqs = []
            for m in range(HC):
                if pre_strips is not None and m < len(pre_strips):
                    strip = pre_strips[m]
                else:
                    strip = sE.tile([P, QC, P], DT, tag="w", bufs=2,
                                    name="wo_strip")
                    eng = nc.sync if m % 2 == 0 else nc.gpsimd
                    eng.dma_start(out=strip, in_=dram["wo"][m][:, :, :P])
                ps = psA.tile([P, FD], mybir.dt.float32, tag="pmm",
                              name="ps_o")
                for kc in range(QC):
                    nc.tensor.matmul(ps, strip[:, kc, :], ctxT[:, kc, sl],
                                     start=(kc == 0), stop=(kc == QC - 1))
                xo = sE.tile([P, FD], mybir.dt.float32, tag="xo", bufs=2,
                             name="xo")
                nc.gpsimd.dma_start(out=xo, in_=xT_v[:, m, sl])
                nc.vector.scalar_tensor_tensor(
                    out=res2[:, m, :], in0=ps, scalar=0.0,
                    in1=xo, op0=ALU.bypass, op1=ALU.add)
                sq = sE.tile([P, FD], DT, tag="sq", bufs=16, name="sq2")
                nc.scalar.activation(sq, res2[:, m, :], AF.Square)
                sqs.append(sq)
                if next_pre is not None and m == HC - 2:
                    # prefetch the first o_proj strips of the next token tile
                    for j, t in enumerate(next_pre):
                        nc.sync.dma_start(out=t, in_=dram["wo"][j][:, :, :P])
            ssum = psB.tile([1, FD], mybir.dt.float32, tag="psmall",
                            name="ps_ss2")
            for m in range(HC):
                nc.tensor.matmul(ssum, ones_bf, sqs[m],
                                 start=(m == 0), stop=(m == HC - 1))
            rr = sE.tile([1, FD], DT_R, tag="rr", name="rr2")
            nc.scalar.activation(rr, ssum, AF.Sqrt, bias=eps_t,
                                 scale=1.0 / HIDDEN)
            rrb = sE.tile([P, FD], mybir.dt.float32, tag="rrb", name="rrb2")
            recip_bcast(rr, rrb, "rms2")
            for kc in range(HC):
                nc.vector.scalar_tensor_tensor(
                    out=h2[nt][:, kc, :], in0=res2[:, kc, :],
                    scalar=g2[:, kc:kc + 1], in1=rrb,
                    op0=ALU.mult, op1=ALU.mult)
            nc.gpsimd.dma_start(out=res2T_v[:, :, sl], in_=res2)

        # ---------------- stage F: SwiGLU MLP + final residual
        def f_strip(w, im, q="sync"):
            s = sF.tile([P, HC, P], DT, tag="w", bufs=3,
                        name=f"{w}_strip")
            eng = nc.sync if q == "sync" else nc.gpsimd
            eng.dma_start(out=s, in_=dram[w][im][:, :, :P])
            return s

        def f_gate_up(tt, pre):
            sl = slice(tt * FD, (tt + 1) * FD)
            act = sF.tile([P, IC, FD], DT, tag="act", bufs=1, name="act")
            for im in range(IC):
                if pre is not None and im == 0:
                    gstrip, ustrip = pre
                else:
                    gstrip = f_strip("wg", im)
                    ustrip = f_strip("wu", im, q="gpsimd")
                ps_g = psA.tile([P, FD], mybir.dt.float32, tag="pmm",
                                name="ps_g")
                for kc in range(HC):
                    nc.tensor.matmul(ps_g, gstrip[:, kc, :],
                                     h2[tt][:, kc, :],
                                     start=(kc == 0), stop=(kc == HC - 1))
                sil = sF.tile([P, FD], mybir.dt.float32, tag="sil",
                              name="sil")
                nc.scalar.activation(sil, ps_g, AF.Sigmoid)
                nc.vector.tensor_mul(sil, sil, ps_g)
                ps_u = psA.tile([P, FD], mybir.dt.float32, tag="pmm",
                                name="ps_u")
                for kc in range(HC):
                    nc.tensor.matmul(ps_u, ustrip[:, kc, :],
                                     h2[tt][:, kc, :],
                                     start=(kc == 0), stop=(kc == HC - 1))
                nc.vector.tensor_mul(act[:, im, :], sil, ps_u)
            return act

        def f_down(tt, act):
            sl = slice(tt * FD, (tt + 1) * FD)
            for dm in range(HC):
                dstrip = sF.tile([P, IC, P], DT, tag="wd", bufs=2,
                                 name="wd_strip")
                nc.gpsimd.dma_start(out=dstrip, in_=dram["wd"][dm][:, :, :P])
                ps_d = psA.tile([P, FD], mybir.dt.float32, tag="pmm",
                                name="ps_d")
                for kc in range(IC):
                    nc.tensor.matmul(ps_d, dstrip[:, kc, :],
                                     act[:, kc, :],
                                     start=(kc == 0), stop=(kc == IC - 1))
                r2c = sF.tile([P, FD], DT, tag="r2c", bufs=1, name="r2c")
                nc.sync.dma_start(out=r2c, in_=res2T_v[:, dm, sl])
                ot = sF.tile([P, FD], mybir.dt.float32, tag="ot",
                             name="ot")
                nc.vector.tensor_add(ot, ps_d, r2c)
                nc.sync.dma_start(out=outT_v[:, dm, sl], in_=ot)

        pre_e1 = [sE.tile([P, QC, P], DT, tag="wopre", bufs=1,
                          name="wo_pre0")]
        stage_e(0, next_pre=pre_e1)
        stage_e(1, pre_strips=pre_e1)
        poolDE.release()
        sF = tc.alloc_tile_pool(name="stF", bufs=2, side="right")
        act0 = f_gate_up(0, None)
        # emit next tile's first gate/up strip loads before the down loop so
        # they sit ahead of the r2c loads in the sync DMA FIFO
        pre1 = (f_strip("wg", 0), f_strip("wu", 0))
        f_down(0, act0)
        act1 = f_gate_up(1, pre1)
        f_down(1, act1)
        sF.release()
        sE.release()
        poolEF.release()


def _prep_inputs(x, pos_ids, time_emb, ln1_w, ln1_tw, ln2_w, ln2_tw,
                 Wq, Wk, Wv, Wo, Wg, Wu, Wd):
    """Host-side layout prep. Returns list of per-core in_maps."""
    shared = {
        "wq": _strips(Wq.T, HC, QC),
        "wk": _strips(Wk.T, HC, KC_HD),
        "wv": np.ascontiguousarray(Wv.T.reshape(HC, P, HD)).astype(BF16),
        "wo": _strips(Wo.T, QC, HC),
        "wg": _strips(Wg.T, HC, IC),
        "wu": _strips(Wu.T, HC, IC),
        "wd": _strips(Wd.T, IC, HC),
    }
    inv_freq = 1.0 / (ROPE_BASE **
                      (np.arange(0, HD, 2, dtype=np.float64) / HD))
    # adaptive gains on host (tiny matvec): g = w * (1 + t @ tw.T)
    t_all = np.asarray(time_emb).astype(F32)                    # (B, H)
    gain1 = (np.asarray(ln1_w) * (1.0 + t_all @ np.asarray(ln1_tw).T.astype(F32)))
    gain2 = (np.asarray(ln2_w) * (1.0 + t_all @ np.asarray(ln2_tw).T.astype(F32)))
    in_maps = []
    for c in range(N_CORES):
        b, half = c // 2, c % 2
        perm = np.r_[np.arange(half * T_OWN, (half + 1) * T_OWN),
                     np.arange((1 - half) * T_OWN, (2 - half) * T_OWN)]
        xTb = np.ascontiguousarray(np.asarray(x[b]).T[:, perm]).astype(F32)
        ang = (np.asarray(pos_ids[b])[perm].astype(np.float64)[:, None]
               * inv_freq[None, :])
        m = dict(shared)
        m["xT"] = xTb
        m["xTb"] = xTb.astype(BF16)
        m["cosT"] = np.ascontiguousarray(np.cos(ang).T).astype(BF16)
        m["sinT"] = np.ascontiguousarray(np.sin(ang).T).astype(BF16)
        m["g1"] = np.ascontiguousarray(gain1[b].reshape(HC, P).T).astype(F32)
        m["g2"] = np.ascontiguousarray(gain2[b].reshape(HC, P).T).astype(F32)
        in_maps.append(m)
    return in_maps


def kernel(**inputs):
    global LAST_RESULTS
    from concourse.bass_utils import run_bass_kernel_spmd

    nc = build_program()
    in_maps = _prep_inputs(**{k: np.asarray(v) for k, v in inputs.items()})
    trace = bool(int(os.environ.get("KERNEL_TRACE", "0")))
    kw = {}
    if os.environ.get("KERNEL_TMPDIR"):
        os.makedirs(os.environ["KERNEL_TMPDIR"], exist_ok=True)
        kw["tmpdir"] = os.environ["KERNEL_TMPDIR"]
    res = run_bass_kernel_spmd(nc, in_maps, core_ids=list(range(N_CORES)),
                               trace=trace, **kw)
    LAST_RESULTS = res
    out = np.empty((B, S, HIDDEN), dtype=F32)
    for c in range(N_CORES):
        b, half = c // 2, c % 2
        out[b, half * T_OWN:(half + 1) * T_OWN, :] = res.results[c]["outT"].T
    return out


# revision 58
# speedup vs baseline: 1.0648x; 1.0138x over previous
"""Trainium2 Bass kernel for nn_ExpertAdaRMSLayer (AdaRMS transformer layer).

Sharding: 8 cores = 4 batches (DP) x 2 token-halves. Each core computes its
1024 tokens end-to-end with no collectives; k/v (nkv=1) are computed
redundantly by the pair of cores sharing a batch. All activations are kept
feature-major [feature, token] on device; the host pre-transposes inputs /
weights and re-assembles the output. Columns are rolled per core so "own"
tokens are always columns 0..1023 (keeps the SPMD program uniform; attention
is permutation-invariant over keys).

v3: adaptive gains g1/g2 precomputed on host. Reciprocal paths via PE
ones-broadcast matmul + full-width DVE reciprocal. Stage order interleaves
the serial rms chains (ACT squares -> sqrt -> bcast -> recip -> STT) under
the dense matmul streams of the next stage: B(0) B(1) Wq(0) B(2) Wq(1) B(3)
Wk Wv rope, and o_proj is token-tile-outer so rms2(nt) hides under
o_proj(nt+1) and the MLP.
"""

import os
import sys
from contextlib import ExitStack

import numpy as np

sys.path.insert(0, "/opt/trn_rl_repo")

import ml_dtypes

import concourse.bass as bass
import concourse.mybir as mybir
import concourse.tile as tile

BF16 = ml_dtypes.bfloat16
F32 = np.float32

# Model dims (hardcoded per spec)
HIDDEN, NQ, NKV, HD, INTER = 2048, 8, 1, 256, 8192
B, S = 4, 2048
EPS = 1e-6
ROPE_BASE = 10000.0

P = 128
HC = HIDDEN // P          # 16 hidden chunks
IC = INTER // P           # 64 inter chunks
QC = (NQ * HD) // P       # 16 q-feature chunks
KC_HD = HD // P           # 2 head-dim chunks
T_OWN = S // 2            # 1024 own tokens per core
T_FULL = S                # 2048 tokens per batch
FD = 512                  # matmul free-dim tile (one PSUM bank of f32)
NT_OWN = T_OWN // FD      # 2
NT_FULL = T_FULL // FD    # 4
N_CORES = 8

DT = mybir.dt.bfloat16    # matmul operand dtype
DT_R = mybir.dt.float32r  # full-rate fp32 dtype for rms sum-of-squares
AF = mybir.ActivationFunctionType
ALU = mybir.AluOpType

_CACHE = {}
LAST_RESULTS = None


PADW = 128  # contiguous strip rows: kc/elem dims merge so strip DMAs ride
             # the large-packet DIRECT2D path (the old 132-pad forced 256B
             # generic packets, capping aggregate DMA at ~95GB/s; multi-wait
             # DMAs are handled by _split_dma_waits)


def _strips(WT, KC, MC):
    """WT: [K, M] f32 with rows = contraction dim. Returns bf16 array
    [MC, 128, KC, PADW] with [m][p][kc][:128] = WT[kc*128+p, m*128+j]."""
    K, M = WT.shape
    assert K == KC * P and M == MC * P
    A = WT.reshape(KC, P, MC, P).transpose(2, 1, 0, 3)
    out = np.zeros((MC, P, KC, PADW), dtype=BF16)
    out[:, :, :, :P] = A.astype(BF16)
    return out


def build_program():
    if "nc" in _CACHE:
        return _CACHE["nc"]

    nc = bass.Bass()
    dram = {}

    def inp(name, shape, dt):
        dram[name] = nc.declare_dram_parameter(name, list(shape), dt,
                                               isOutput=False)

    inp("xT", (HIDDEN, T_FULL), mybir.dt.float32)
    inp("xTb", (HIDDEN, T_FULL), DT)
    inp("cosT", (P, T_FULL), DT)
    inp("sinT", (P, T_FULL), DT)
    inp("g1", (P, HC), mybir.dt.float32)
    inp("g2", (P, HC), mybir.dt.float32)
    inp("wq", (QC, P, HC, PADW), DT)
    inp("wk", (KC_HD, P, HC, PADW), DT)
    inp("wv", (HC, P, HD), DT)
    inp("wo", (HC, P, QC, PADW), DT)
    inp("wg", (IC, P, HC, PADW), DT)
    inp("wu", (IC, P, HC, PADW), DT)
    inp("wd", (HC, P, IC, PADW), DT)
    outT = nc.declare_dram_parameter("outT", [HIDDEN, T_OWN],
                                     mybir.dt.float32, isOutput=True)
    if os.environ.get("KERNEL_DEBUG_DUMP"):
        for nm, shp in (("dbg_h1T", [HIDDEN, T_FULL]), ("dbg_qT", [NQ * HD, T_OWN]),
                        ("dbg_kT", [HD, T_FULL]), ("dbg_v", [T_FULL, HD]),
                        ("dbg_ctxT", [NQ * HD, T_OWN])):
            dram[nm] = nc.dram_tensor(nm, shp, mybir.dt.float32)
    res2T = nc.dram_tensor("res2T", [HIDDEN, T_OWN], DT)

    _build_kernel(nc, dram, outT, res2T)
    if not os.environ.get("KERNEL_NO_WAIT_SPLIT"):
        _split_dma_waits(nc)
    _CACHE["nc"] = nc
    return nc


def _split_dma_waits(nc):
    """This walrus encodes at most ONE sync-wait per instruction (the ISA
    EVENTS struct has a single wait slot and this build refuses to split).
    Hoist all waits of multi-wait instructions onto standalone
    event-semaphore instructions on the issuing engine/sequencer, which
    executes them in program order before the original instruction."""
    n = 0
    for f in nc.m.functions:
        for bb in f.blocks:
            out = []
            changed = False
            for inst in bb.instructions:
                si = inst.sync_info
                if si is not None and len(si.on_wait) > 1:
                    for w in si.on_wait:
                        ev = mybir.InstEventSemaphore(
                            name=f"{inst.name}_w{n}", ins=[], outs=[])
                        ev.engine = inst.engine
                        ev.sync_info = mybir.SyncInfo(on_wait=[w],
                                                      on_update=[])
                        out.append(ev)
                        n += 1
                    inst.sync_info = mybir.SyncInfo(
                        on_wait=[], on_update=list(si.on_update))
                    changed = True
                out.append(inst)
            if changed:
                bb.instructions[:] = out
    return n


def _build_kernel(nc, dram, outT, res2T):
    xT_v = dram["xT"][:, :].rearrange("(c p) t -> p c t", p=P)
    xTb_v = dram["xTb"][:, :].rearrange("(c p) t -> p c t", p=P)
    res2T_v = res2T[:, :].rearrange("(c p) t -> p c t", p=P)
    outT_v = outT[:, :].rearrange("(c p) t -> p c t", p=P)

    with tile.TileContext(nc) as tc, ExitStack() as top:
        const = top.enter_context(tc.tile_pool(name="const", bufs=1))
        psA = top.enter_context(tc.tile_pool(name="psA", bufs=6, space="PSUM"))
        psB = top.enter_context(tc.tile_pool(name="psB", bufs=2, space="PSUM"))

        ones_bf = const.tile([P, 1], DT)
        nc.vector.memset(ones_bf, 1.0)
        ones_rf = const.tile([P, 1], mybir.dt.float32, name="ones_rf")
        nc.vector.memset(ones_rf, 1.0)
        ones_r = ones_rf.bitcast(DT_R)
        ones_row_f = const.tile([1, P], mybir.dt.float32, name="ones_row_f")
        nc.vector.memset(ones_row_f, 1.0)
        ones_row_r = ones_row_f.bitcast(DT_R)
        g1 = const.tile([P, HC], mybir.dt.float32, name="g1")
        g2 = const.tile([P, HC], mybir.dt.float32, name="g2")
        nc.sync.dma_start(out=g1, in_=dram["g1"][:, :])
        nc.sync.dma_start(out=g2, in_=dram["g2"][:, :])
        eps_t = const.tile([1, 1], mybir.dt.float32, name="eps_t")
        nc.vector.memset(eps_t, EPS)
        # PE warm-up: dense dummy matmul burst at t=0 so HAM un-throttles
        # before the first real matmul stream arrives.
        wu_w = const.tile([P, P], DT, name="wu_w")
        nc.vector.memset(wu_w, 0.0)
        wu_x = const.tile([P, FD], DT, name="wu_x")
        nc.vector.memset(wu_x, 0.0)
        wu_ps = psA.tile([P, FD], mybir.dt.float32, tag="pmm", name="ps_wu")
        NWU = 48
        for i in range(NWU):
            nc.tensor.matmul(wu_ps, wu_w, wu_x,
                             start=(i == 0), stop=(i == NWU - 1))

        def recip_bcast(sq_row, rec_out, tag):
            """sq_row: SBUF [1, FD] f32r (already sqrt'ed or raw denom).
            Broadcasts across 128 partitions via a f32r ones-matmul, then
            full-width DVE reciprocal into rec_out (SBUF [P, FD] f32)."""
            bc = psB.tile([P, FD], mybir.dt.float32, tag="psmall",
                          name=f"ps_bc_{tag}")
            nc.tensor.matmul(bc, ones_row_r, sq_row,
                             start=True, stop=True)
            nc.vector.reciprocal(rec_out, bc)

        # ---------------- stage B: ada_rms1 -> h1 (bf16), one token-tile
        poolBC = tc.alloc_tile_pool(name="poolBC", bufs=1)
        h1 = [poolBC.tile([P, HC, FD], DT, name=f"h1_{nt}")
              for nt in range(NT_FULL)]
        sB = tc.alloc_tile_pool(name="stB", bufs=2)

        def b_dma(nt):
            sl = slice(nt * FD, (nt + 1) * FD)
            xt = sB.tile([P, HC, FD], DT, tag="xt", name="x_tile")
            for kc in range(HC):
                nc.gpsimd.dma_start(out=xt[:, kc, :], in_=xTb_v[:, kc, sl])
            return xt

        def b_square_thunks(xt, sqs):
            """Per-chunk square ops as thunks so their emission can be
            interleaved into another block's ACT stream (avoids a 16-op
            head-of-line burst in front of that block's PSUM evacuations)."""
            def mk(kc):
                def go():
                    sq = sB.tile([P, FD], DT, tag="sq", bufs=13, name="sq")
                    nc.scalar.activation(sq, xt[:, kc, :], AF.Square)
                    sqs.append(sq)
                return go
            return [mk(kc) for kc in range(HC)]

        def b_reduce(nt, xt, sqs):
            """Batch ssum matmuls (inputs already computed -> no PE FIFO
            stall), then rms tail and h1 writeback."""
            ssum = psB.tile([1, FD], mybir.dt.float32, tag="psmall",
                            name="ps_ss")
            for kc in range(HC):
                nc.tensor.matmul(ssum, ones_bf, sqs[kc],
                                 start=(kc == 0), stop=(kc == HC - 1))
            rr = sB.tile([1, FD], DT_R, tag="rr", name="rr")
            nc.scalar.activation(rr, ssum, AF.Sqrt, bias=eps_t,
                                 scale=1.0 / HIDDEN)
            rrb = sB.tile([P, FD], mybir.dt.float32, tag="rrb", name="rrb")
            recip_bcast(rr, rrb, "rms1")
            for kc in range(HC):
                nc.vector.scalar_tensor_tensor(
                    out=h1[nt][:, kc, :], in0=xt[:, kc, :],
                    scalar=g1[:, kc:kc + 1], in1=rrb,
                    op0=ALU.mult, op1=ALU.mult)

        # ---------------- stage C pieces: QKV + RoPE
        poolCD = tc.alloc_tile_pool(name="poolCD", bufs=1, side="right")
        qT = poolCD.tile([P, QC, T_OWN], DT, name="qT")
        kT = poolCD.tile([P, KC_HD, T_FULL], DT, name="kT")
        vtok = poolCD.tile([P, HC, HD], DT, name="vtok")
        sC = tc.alloc_tile_pool(name="stC", bufs=3)

        def emit_some(thunks, n):
            for _ in range(n):
                if thunks:
                    thunks.pop(0)()

        def wq_tile(nt, filler=None):
            sl = slice(nt * FD, (nt + 1) * FD)
            for m in range(QC):
                strip = sC.tile([P, HC, P], DT, tag="w", bufs=3, name="wq_strip")
                eng = nc.sync if m % 2 == 0 else nc.gpsimd
                eng.dma_start(out=strip, in_=dram["wq"][m][:, :, :P])
                ps = psA.tile([P, FD], mybir.dt.float32, tag="pmm",
                              name="ps_q")
                for kc in range(HC):
                    nc.tensor.matmul(ps, strip[:, kc, :], h1[nt][:, kc, :],
                                     start=(kc == 0), stop=(kc == HC - 1))
                nc.scalar.copy(out=qT[:, m, sl], in_=ps)
                if filler:
                    emit_some(filler, 1)

        def wk_tile(nts, filler=None):
            for m in range(KC_HD):
                for nt in nts:
                    strip = sC.tile([P, HC, P], DT, tag="w", bufs=3, name="wk_strip")
                    eng = nc.sync if nt % 2 == 0 else nc.gpsimd
                    eng.dma_start(out=strip, in_=dram["wk"][m][:, :, :P])
                    sl = slice(nt * FD, (nt + 1) * FD)
                    ps = psA.tile([P, FD], mybir.dt.float32, tag="pmm",
                                  name="ps_k")
                    for kc in range(HC):
                        nc.tensor.matmul(ps, strip[:, kc, :], h1[nt][:, kc, :],
                                         start=(kc == 0), stop=(kc == HC - 1))
                    nc.scalar.copy(out=kT[:, m, sl], in_=ps)
                    if filler:
                        emit_some(filler, 3)

        cos_f = poolCD.tile([P, T_FULL], DT, name="cos_f")
        sin_f = poolCD.tile([P, T_FULL], DT, name="sin_f")
        nc.gpsimd.dma_start(out=cos_f, in_=dram["cosT"][:, :])
        nc.gpsimd.dma_start(out=sin_f, in_=dram["sinT"][:, :])

        def rope_pair(u, v_, cos_t, sin_t, width):
            t1 = sC.tile([P, width], DT, tag="rt1", bufs=1, name="rope_t1")
            t2 = sC.tile([P, width], DT, tag="rt2", bufs=1, name="rope_t2")
            t3 = sC.tile([P, width], DT, tag="rt3", bufs=1, name="rope_t3")
            nc.vector.tensor_mul(t1, u, sin_t)      # u*sin
            nc.vector.tensor_mul(t2, u, cos_t)      # u*cos
            nc.vector.tensor_mul(t3, v_, sin_t)     # v*sin
            nc.vector.tensor_sub(u, t2, t3)         # u <- u*cos - v*sin
            nc.vector.tensor_mul(t2, v_, cos_t)     # v*cos
            nc.vector.tensor_add(v_, t2, t1)        # v <- v*cos + u*sin

        def wv_rope():
            # RoPE kT first (DVE) so the vtok matmul stream hides its latency
            rope_pair(kT[:, 0, :], kT[:, 1, :], cos_f, sin_f, T_FULL)
            # v token-major: [key-token-in-chunk, key-chunk, hd]
            wv_sb = poolCD.tile([P, HC, HD], DT, name="wv_sb")
            nc.sync.dma_start(out=wv_sb, in_=dram["wv"][:, :, :].rearrange("c p d -> p c d"))
            for tm in range(T_FULL // P):
                ps = psA.tile([P, HD], mybir.dt.float32, tag="pmm",
                              name="ps_v")
                nt, tin = (tm * P) // FD, (tm * P) % FD
                tsl = slice(tin, tin + P)
                for kc in range(HC):
                    nc.tensor.matmul(ps, h1[nt][:, kc, tsl], wv_sb[:, kc, :],
                                     start=(kc == 0), stop=(kc == HC - 1))
                nc.scalar.copy(out=vtok[:, tm, :], in_=ps)

            for h in range(NQ):
                rope_pair(qT[:, 2 * h, :], qT[:, 2 * h + 1, :],
                          cos_f[:, :T_OWN], sin_f[:, :T_OWN], T_OWN)
            if "dbg_h1T" in dram:
                dv = dram["dbg_h1T"][:, :].rearrange("(c p) t -> p c t", p=P)
                for nt in range(NT_FULL):
                    nc.gpsimd.dma_start(
                        out=dv[:, :, nt * FD:(nt + 1) * FD], in_=h1[nt])
                dv = dram["dbg_qT"][:, :].rearrange("(c p) t -> p c t", p=P)
                nc.gpsimd.dma_start(out=dv, in_=qT)
                dv = dram["dbg_kT"][:, :].rearrange("(c p) t -> p c t", p=P)
                nc.gpsimd.dma_start(out=dv, in_=kT)
                dv = dram["dbg_v"][:, :].rearrange("(c p) t -> p c t", p=P)
                nc.gpsimd.dma_start(out=dv, in_=vtok)

        # interleave: rms loads/squares overlap the Wq matmul blocks; the
        # batch ssum-reduce matmuls are emitted only after a dense block so
        # their inputs are ready when the PE FIFO reaches them. Squares of
        # tile nt+1 are interleaved into tile nt's evacuation stream on ACT.
        xt0 = b_dma(nt=0)
        sq0 = []
        emit_some(b_square_thunks(xt0, sq0), HC)
        b_reduce(0, xt0, sq0)
        xt1 = b_dma(1)
        sq1 = []
        wq_tile(0, filler=b_square_thunks(xt1, sq1))
        b_reduce(1, xt1, sq1)
        xt2 = b_dma(2)
        sq2 = []
        wq_tile(1, filler=b_square_thunks(xt2, sq2))
        b_reduce(2, xt2, sq2)
        xt3 = b_dma(3)
        sq3 = []
        wk_tile([0, 1, 2], filler=b_square_thunks(xt3, sq3))
        b_reduce(3, xt3, sq3)
        wk_tile([3])
        wv_rope()
        sC.release()
        sB.release()
        poolBC.release()

        # ---------------- stage D: attention (column softmax, no transposes)
        poolDE = tc.alloc_tile_pool(name="poolDE", bufs=1)
        ctxT = poolDE.tile([P, QC, T_OWN], DT, name="ctxT")
        with tc.tile_pool(name="stD", bufs=2) as sD:
            for h in range(NQ):
                attnT = sD.tile([P, HC, T_OWN], DT, tag="attn", name="attnT")
                for sm in range(T_FULL // P):
                    for nt in range(NT_OWN):
                        sl = slice(nt * FD, (nt + 1) * FD)
                        ps = psA.tile([P, FD], mybir.dt.float32, tag="pmm",
                                      name="ps_sc")
                        for dc in range(KC_HD):
                            nc.tensor.matmul(
                                ps, kT[:, dc, sm * P:(sm + 1) * P],
                                qT[:, 2 * h + dc, sl],
                                start=(dc == 0), stop=(dc == KC_HD - 1))
                        nc.scalar.activation(attnT[:, sm, sl], ps, AF.Exp,
                                             scale=1.0 / 16.0)
                for nt in range(NT_OWN):
                    sl = slice(nt * FD, (nt + 1) * FD)
                    cs = psB.tile([1, FD], mybir.dt.float32, tag="psmall",
                                  name="ps_cs")
                    for kc in range(HC):
                        nc.tensor.matmul(cs, ones_bf, attnT[:, kc, sl],
                                         start=(kc == 0), stop=(kc == HC - 1))
                    cs_sb = sD.tile([1, FD], DT_R, tag="cs_sb",
                                    name="cs_sb")
                    nc.scalar.copy(out=cs_sb, in_=cs)
                    recb = sD.tile([P, FD], mybir.dt.float32, tag="recb",
                                   name="recb")
                    # ctx dm=0 keeps PE busy while the ACT copy + bcast for
                    # the softmax denominator run; then dm=1.
                    pss = []
                    for dm in range(KC_HD):
                        if dm == 1:
                            recip_bcast(cs_sb, recb, "attn")
                        ps = psA.tile([P, FD], mybir.dt.float32, tag="pmm",
                                      name="ps_ctx")
                        for kc in range(HC):
                            nc.tensor.matmul(
                                ps, vtok[:, kc, dm * P:(dm + 1) * P],
                                attnT[:, kc, sl],
                                start=(kc == 0), stop=(kc == HC - 1))
                        pss.append(ps)
                    for dm in range(KC_HD):
                        nc.vector.tensor_mul(ctxT[:, 2 * h + dm, sl],
                                             pss[dm], recb)
            if "dbg_ctxT" in dram:
                dv = dram["dbg_ctxT"][:, :].rearrange("(c p) t -> p c t", p=P)
                nc.gpsimd.dma_start(out=dv, in_=ctxT)
        poolCD.release()

        # ---------------- stage E: o_proj + residual + ada_rms2 -> h2
        # token-tile-outer so the rms2(nt) chain hides under o_proj(nt+1)
        # and the MLP matmul stream.
        poolEF = tc.alloc_tile_pool(name="poolEF", bufs=1, side="right")
        h2 = [poolEF.tile([P, HC, FD], DT, name=f"h2_{nt}")
              for nt in range(NT_OWN)]
        sE = tc.alloc_tile_pool(name="stE", bufs=2, side="right")

        def stage_e(nt, pre_strips=None, next_pre=None):
            sl = slice(nt * FD, (nt + 1) * FD)
            res2 = sE.tile([P, HC, FD], DT, tag="res2", bufs=1,
                           name=f"res2_{nt}")
            sqs = []
            for m in range(HC):
                if pre_strips is not None and m < len(pre_strips):
                    strip = pre_strips[m]
                else:
                    strip = sE.tile([P, QC, P], DT, tag="w", bufs=2,
                                    name="wo_strip")
                    eng = nc.sync if m % 2 == 0 else nc.gpsimd
                    eng.dma_start(out=strip, in_=dram["wo"][m][:, :, :P])
                ps = psA.tile([P, FD], mybir.dt.float32, tag="pmm",
                              name="ps_o")
                for kc in range(QC):
                    nc.tensor.matmul(ps, strip[:, kc, :], ctxT[:, kc, sl],
                                     start=(kc == 0), stop=(kc == QC - 1))
                xo = sE.tile([P, FD], DT, tag="xo", bufs=2, name="xo")
                nc.gpsimd.dma_start(out=xo, in_=xTb_v[:, m, sl])
                nc.vector.scalar_tensor_tensor(
                    out=res2[:, m, :], in0=ps, scalar=0.0,
                    in1=xo, op0=ALU.bypass, op1=ALU.add)
                sq = sE.tile([P, FD], DT, tag="sq", bufs=16, name="sq2")
                nc.scalar.activation(sq, res2[:, m, :], AF.Square)
                sqs.append(sq)
                if next_pre is not None and m == HC - 2:
                    # prefetch the first o_proj strips of the next token tile
                    for j, t in enumerate(next_pre):
                        nc.sync.dma_start(out=t, in_=dram["wo"][j][:, :, :P])
            ssum = psB.tile([1, FD], mybir.dt.float32, tag="psmall",
                            name="ps_ss2")
            for m in range(HC):
                nc.tensor.matmul(ssum, ones_bf, sqs[m],
                                 start=(m == 0), stop=(m == HC - 1))
            rr = sE.tile([1, FD], DT_R, tag="rr", name="rr2")
            nc.scalar.activation(rr, ssum, AF.Sqrt, bias=eps_t,
                                 scale=1.0 / HIDDEN)
            rrb = sE.tile([P, FD], mybir.dt.float32, tag="rrb", name="rrb2")
            recip_bcast(rr, rrb, "rms2")
            for kc in range(HC):
                nc.vector.scalar_tensor_tensor(
                    out=h2[nt][:, kc, :], in0=res2[:, kc, :],
                    scalar=g2[:, kc:kc + 1], in1=rrb,
                    op0=ALU.mult, op1=ALU.mult)
            nc.gpsimd.dma_start(out=res2T_v[:, :, sl], in_=res2)

        # ---------------- stage F: SwiGLU MLP + final residual
        def f_strip(w, im, q="sync"):
            s = sF.tile([P, HC, P], DT, tag="w", bufs=3,
                        name=f"{w}_strip")
            eng = nc.sync if q == "sync" else nc.gpsimd
            eng.dma_start(out=s, in_=dram[w][im][:, :, :P])
            return s

        def f_gate_up(tt, pre):
            sl = slice(tt * FD, (tt + 1) * FD)
            act = sF.tile([P, IC, FD], DT, tag="act", bufs=1, name="act")
            for im in range(IC):
                if pre is not None and im == 0:
                    gstrip, ustrip = pre
                else:
                    gstrip = f_strip("wg", im)
                    ustrip = f_strip("wu", im, q="gpsimd")
                ps_g = psA.tile([P, FD], mybir.dt.float32, tag="pmm",
                                name="ps_g")
                for kc in range(HC):
                    nc.tensor.matmul(ps_g, gstrip[:, kc, :],
                                     h2[tt][:, kc, :],
                                     start=(kc == 0), stop=(kc == HC - 1))
                sil = sF.tile([P, FD], mybir.dt.float32, tag="sil",
                              name="sil")
                nc.scalar.activation(sil, ps_g, AF.Sigmoid)
                nc.vector.tensor_mul(sil, sil, ps_g)
                ps_u = psA.tile([P, FD], mybir.dt.float32, tag="pmm",
                                name="ps_u")
                for kc in range(HC):
                    nc.tensor.matmul(ps_u, ustrip[:, kc, :],
                                     h2[tt][:, kc, :],
                                     start=(kc == 0), stop=(kc == HC - 1))
                nc.vector.tensor_mul(act[:, im, :], sil, ps_u)
            return act

        def f_down(tt, act):
            sl = slice(tt * FD, (tt + 1) * FD)
            IH = IC // 2
            for dm in range(HC):
                da = sF.tile([P, IH, P], DT, tag="wda", bufs=2,
                             name="wd_a")
                nc.gpsimd.dma_start(out=da, in_=dram["wd"][dm][:, :IH, :P])
                db = sF.tile([P, IH, P], DT, tag="wdb", bufs=2,
                             name="wd_b")
                nc.sync.dma_start(out=db, in_=dram["wd"][dm][:, IH:, :P])
                halves = (da, db)
                ps_d = psA.tile([P, FD], mybir.dt.float32, tag="pmm",
                                name="ps_d")
                for kc in range(IC):
                    nc.tensor.matmul(ps_d, halves[kc // IH][:, kc % IH, :],
                                     act[:, kc, :],
                                     start=(kc == 0), stop=(kc == IC - 1))
                r2c = sF.tile([P, FD], DT, tag="r2c", bufs=1, name="r2c")
                nc.sync.dma_start(out=r2c, in_=res2T_v[:, dm, sl])
                ot = sF.tile([P, FD], mybir.dt.float32, tag="ot",
                             name="ot")
                nc.vector.tensor_add(ot, ps_d, r2c)
                nc.sync.dma_start(out=outT_v[:, dm, sl], in_=ot)

        pre_e1 = [sE.tile([P, QC, P], DT, tag="wopre", bufs=1,
                          name="wo_pre0")]
        stage_e(0, next_pre=pre_e1)
        stage_e(1, pre_strips=pre_e1)
        poolDE.release()
        sF = tc.alloc_tile_pool(name="stF", bufs=2, side="right")
        act0 = f_gate_up(0, None)
        # emit next tile's first gate/up strip loads before the down loop so
        # they sit ahead of the r2c loads in the sync DMA FIFO
        pre1 = (f_strip("wg", 0), f_strip("wu", 0))
        f_down(0, act0)
        act1 = f_gate_up(1, pre1)
        f_down(1, act1)
        sF.release()
        sE.release()
        poolEF.release()


def _prep_inputs(x, pos_ids, time_emb, ln1_w, ln1_tw, ln2_w, ln2_tw,
                 Wq, Wk, Wv, Wo, Wg, Wu, Wd):
    """Host-side layout prep. Returns list of per-core in_maps."""
    shared = {
        "wq": _strips(Wq.T, HC, QC),
        "wk": _strips(Wk.T, HC, KC_HD),
        "wv": np.ascontiguousarray(Wv.T.reshape(HC, P, HD)).astype(BF16),
        "wo": _strips(Wo.T, QC, HC),
        "wg": _strips(Wg.T, HC, IC),
        "wu": _strips(Wu.T, HC, IC),
        "wd": _strips(Wd.T, IC, HC),
    }
    inv_freq = 1.0 / (ROPE_BASE **
                      (np.arange(0, HD, 2, dtype=np.float64) / HD))
    # adaptive gains on host (tiny matvec): g = w * (1 + t @ tw.T)
    t_all = np.asarray(time_emb).astype(F32)                    # (B, H)
    gain1 = (np.asarray(ln1_w) * (1.0 + t_all @ np.asarray(ln1_tw).T.astype(F32)))
    gain2 = (np.asarray(ln2_w) * (1.0 + t_all @ np.asarray(ln2_tw).T.astype(F32)))
    in_maps = []
    for c in range(N_CORES):
        b, half = c // 2, c % 2
        perm = np.r_[np.arange(half * T_OWN, (half + 1) * T_OWN),
                     np.arange((1 - half) * T_OWN, (2 - half) * T_OWN)]
        xTb = np.ascontiguousarray(np.asarray(x[b]).T[:, perm]).astype(F32)
        ang = (np.asarray(pos_ids[b])[perm].astype(np.float64)[:, None]
               * inv_freq[None, :])
        m = dict(shared)
        m["xT"] = xTb
        m["xTb"] = xTb.astype(BF16)
        m["cosT"] = np.ascontiguousarray(np.cos(ang).T).astype(BF16)
        m["sinT"] = np.ascontiguousarray(np.sin(ang).T).astype(BF16)
        m["g1"] = np.ascontiguousarray(gain1[b].reshape(HC, P).T).astype(F32)
        m["g2"] = np.ascontiguousarray(gain2[b].reshape(HC, P).T).astype(F32)
        in_maps.append(m)
    return in_maps


def kernel(**inputs):
    global LAST_RESULTS
    from concourse.bass_utils import run_bass_kernel_spmd

    nc = build_program()
    in_maps = _prep_inputs(**{k: np.asarray(v) for k, v in inputs.items()})
    trace = bool(int(os.environ.get("KERNEL_TRACE", "0")))
    kw = {}
    if os.environ.get("KERNEL_TMPDIR"):
        os.makedirs(os.environ["KERNEL_TMPDIR"], exist_ok=True)
        kw["tmpdir"] = os.environ["KERNEL_TMPDIR"]
    res = run_bass_kernel_spmd(nc, in_maps, core_ids=list(range(N_CORES)),
                               trace=trace, **kw)
    LAST_RESULTS = res
    out = np.empty((B, S, HIDDEN), dtype=F32)
    for c in range(N_CORES):
        b, half = c // 2, c % 2
        out[b, half * T_OWN:(half + 1) * T_OWN, :] = res.results[c]["outT"].T
    return out
